# revision 1
# baseline (speedup 1.0000x reference)
"""RWKV block (LN1 -> time-mix attention w/ WKV scan -> LN2 -> channel-mix FFN)
as a Bass/Tile kernel for 8 Trainium2 NeuronCores.

Sharding: data-parallel over batch B=8 (one batch element per core); weights
replicated. No collectives. Inside a core everything runs channel-major
([C-partitions, T-free]) so the WKV recurrence maps onto DVE tensor_tensor_scan
(one instruction per 128-channel tile scans all T=2048 steps).
"""
import sys
if '/opt/trn_rl_repo' not in sys.path:
    sys.path.insert(0, '/opt/trn_rl_repo')

import os
import numpy as np

B, T, C = 8, 2048, 1024
H = 4 * C
NCO = C // 128          # 8 channel tiles
NHO = H // 128          # 32 hidden tiles
TCH = 512               # matmul free-dim chunk (one PSUM bank)
NT = T // TCH           # 4 chunks
NTT = T // 128          # 16 token tiles
LN_EPS = 1e-5

# per-channel vector slot indices in the packed [C, 12] table
(V_TMA, V_CAA, V_CBA, V_ED, V_EU, V_G1, V_B1, V_G2, V_B2,
 V_TMF, V_CAF, V_CBF) = range(12)

_CACHE = {}


def _build():
    import concourse.bacc as bacc
    import concourse.tile as tile
    import concourse.bass as bass
    from concourse import mybir
    from concourse.masks import make_identity
    from contextlib import ExitStack

    f32 = mybir.dt.float32
    bf16 = mybir.dt.bfloat16
    AF = mybir.ActivationFunctionType
    OP = mybir.AluOpType

    nc = bacc.Bacc("TRN2", num_devices=B)

    x_d = nc.dram_tensor("x", [T, C], f32, kind="ExternalInput").ap()
    wk_d = nc.dram_tensor("wk", [C, C], bf16, kind="ExternalInput").ap()
    wv_d = nc.dram_tensor("wv", [C, C], bf16, kind="ExternalInput").ap()
    wr_d = nc.dram_tensor("wr", [C, C], bf16, kind="ExternalInput").ap()
    wo_d = nc.dram_tensor("wo", [C, C], bf16, kind="ExternalInput").ap()
    fk_d = nc.dram_tensor("fk", [C, H], bf16, kind="ExternalInput").ap()
    fv_d = nc.dram_tensor("fv", [H, C], bf16, kind="ExternalInput").ap()
    fr_d = nc.dram_tensor("fr", [C, C], bf16, kind="ExternalInput").ap()
    pv_d = nc.dram_tensor("pv", [C, 12], f32, kind="ExternalInput").ap()
    y_d = nc.dram_tensor("y", [T, C], f32, kind="ExternalOutput").ap()

    x1t_d = nc.dram_tensor("x1t_scr", [C, T], f32).ap()
    x3t_d = nc.dram_tensor("x3t_scr", [C, T], f32).ap()
    rows_d = nc.dram_tensor("rows_scr", [2, T], f32).ap()

    # channel-major view for the phase-A strided store: [c, co, t]
    x1t_v = x1t_d.rearrange("(co c) t -> c co t", c=128)
    wk_v = wk_d.rearrange("(ci k) m -> k ci m", k=128)
    wv_v = wv_d.rearrange("(ci k) m -> k ci m", k=128)
    wr_v = wr_d.rearrange("(ci k) m -> k ci m", k=128)
    wo_v = wo_d.rearrange("(ci k) m -> k ci m", k=128)
    fk_v = fk_d.rearrange("(ci k) m -> k ci m", k=128)

    with tile.TileContext(nc) as tc, ExitStack() as top:
        singles = top.enter_context(tc.tile_pool(name="singles", bufs=1))
        ident = singles.tile([128, 128], f32)
        make_identity(nc, ident)
        ones_col = singles.tile([128, 1], bf16)
        nc.vector.memset(ones_col, 1.0)
        eps_t = singles.tile([128, 1], f32)
        nc.vector.memset(eps_t, LN_EPS)
        pv_sb = []
        for co in range(NCO):
            pvt = singles.tile([128, 12], f32, tag=f"pv{co}")
            nc.sync.dma_start(out=pvt, in_=pv_d[co * 128:(co + 1) * 128, :])
            pv_sb.append(pvt)

        def pvs(co, idx):
            return pv_sb[co][:, idx:idx + 1]

        pp_mm = top.enter_context(tc.tile_pool(name="pp_mm", bufs=3, space="PSUM"))
        pp_tr = top.enter_context(tc.tile_pool(name="pp_tr", bufs=2, space="PSUM"))
        pp_row = top.enter_context(tc.tile_pool(name="pp_row", bufs=2, space="PSUM"))

        # ---------------- Phase A: LN1 (token-major) + transpose to x1t_d ----
        with ExitStack() as ph:
            pa = ph.enter_context(tc.tile_pool(name="pa", bufs=3))
            for tt in range(NTT):
                xt = pa.tile([128, C], f32, tag="xt")
                nc.sync.dma_start(out=xt, in_=x_d[tt * 128:(tt + 1) * 128, :])
                st = pa.tile([128, 2, 6], f32, tag="st")
                nc.vector.bn_stats(out=st[:, 0, :], in_=xt[:, 0:512])
                nc.vector.bn_stats(out=st[:, 1, :], in_=xt[:, 512:1024])
                mv = pa.tile([128, 2], f32, tag="mv")
                nc.vector.bn_aggr(out=mv, in_=st)
                rs = pa.tile([128, 1], f32, tag="rs")
                nc.scalar.activation(out=rs, in_=mv[:, 1:2], func=AF.Sqrt,
                                     bias=eps_t, scale=1.0)
                nc.vector.reciprocal(out=rs, in_=rs)
                xn = pa.tile([128, C], f32, tag="xn")
                nc.vector.tensor_scalar(out=xn, in0=xt, scalar1=mv[:, 0:1],
                                        scalar2=rs, op0=OP.subtract, op1=OP.mult)
                stg = pa.tile([128, NCO, 128], f32, tag="stg")
                for co in range(NCO):
                    ps = pp_tr.tile([128, 128], f32, tag="tra")
                    nc.tensor.transpose(ps, xn[:, co * 128:(co + 1) * 128], ident)
                    nc.vector.tensor_scalar(out=stg[:, co, :], in0=ps,
                                            scalar1=pvs(co, V_G1), scalar2=pvs(co, V_B1),
                                            op0=OP.mult, op1=OP.add)
                nc.sync.dma_start(out=x1t_v[:, :, tt * 128:(tt + 1) * 128], in_=stg)

        with ExitStack() as s1:
            attbf_p = s1.enter_context(tc.tile_pool(name="attbf", bufs=NCO))
            attbf = []

            # ------------ Phase B: mix -> xmbf; GEMMs k/v/r; WKV scan -> attbf
            with ExitStack() as sB:
                xmbf_p = sB.enter_context(tc.tile_pool(name="xmbf", bufs=NCO))
                xmbf = []
                with ExitStack() as ph:
                    pb = ph.enter_context(tc.tile_pool(name="pb", bufs=2))
                    for co in range(NCO):
                        x1 = pb.tile([128, T], f32, tag="x1")
                        nc.sync.dma_start(out=x1, in_=x1t_d[co * 128:(co + 1) * 128, :])
                        xm = pb.tile([128, T], f32, tag="xm")
                        nc.vector.tensor_scalar_mul(out=xm, in0=x1, scalar1=pvs(co, V_TMA))
                        nc.vector.scalar_tensor_tensor(
                            out=xm[:, 1:T], in0=x1[:, 0:T - 1], scalar=pvs(co, V_CAA),
                            in1=xm[:, 1:T], op0=OP.mult, op1=OP.add)
                        nc.vector.scalar_tensor_tensor(
                            out=xm[:, 0:T - 1], in0=x1[:, 1:T], scalar=pvs(co, V_CBA),
                            in1=xm[:, 0:T - 1], op0=OP.mult, op1=OP.add)
                        xb = xmbf_p.tile([128, T], bf16, tag="xmbf")
                        nc.scalar.copy(out=xb, in_=xm)
                        xmbf.append(xb)

                with ExitStack() as ph:
                    pw = ph.enter_context(tc.tile_pool(name="pw", bufs=2))
                    pw1 = ph.enter_context(tc.tile_pool(name="pw1", bufs=1))
                    for co in range(NCO):
                        wkw = pw.tile([128, NCO, 128], bf16, tag="wkw")
                        wvw = pw.tile([128, NCO, 128], bf16, tag="wvw")
                        wrw = pw.tile([128, NCO, 128], bf16, tag="wrw")
                        csl = slice(co * 128, (co + 1) * 128)
                        nc.sync.dma_start(out=wkw, in_=wk_v[:, :, csl])
                        nc.sync.dma_start(out=wvw, in_=wv_v[:, :, csl])
                        nc.sync.dma_start(out=wrw, in_=wr_v[:, :, csl])
                        kk = pw.tile([128, T], f32, tag="kk")
                        vv = pw.tile([128, T], f32, tag="vv")
                        sr = pw1.tile([128, T], f32, tag="sr")
                        for dst, wsb, act in ((kk, wkw, AF.Exp), (vv, wvw, None),
                                              (sr, wrw, AF.Sigmoid)):
                            for nch in range(NT):
                                ps = pp_mm.tile([128, TCH], f32, tag="mm")
                                for ci in range(NCO):
                                    nc.tensor.matmul(
                                        ps, wsb[:, ci, :],
                                        xmbf[ci][:, nch * TCH:(nch + 1) * TCH],
                                        start=(ci == 0), stop=(ci == NCO - 1))
                                sl = dst[:, nch * TCH:(nch + 1) * TCH]
                                if act is None:
                                    nc.vector.tensor_copy(out=sl, in_=ps)
                                else:
                                    nc.scalar.activation(out=sl, in_=ps, func=act)
                        # WKV: S_a = scan(ed, k*v), S_b = scan(ed, k)
                        edb = pw1.tile([128, T], f32, tag="edb")
                        nc.gpsimd.memset(edb, 1.0)
                        nc.vector.tensor_scalar_mul(out=edb, in0=edb,
                                                    scalar1=pvs(co, V_ED))
                        nc.vector.tensor_mul(out=vv, in0=vv, in1=kk)  # vv := k*v
                        sa = pw1.tile([128, T], f32, tag="sa")
                        nc.vector.tensor_tensor_scan(out=sa, data0=edb, data1=vv,
                                                     initial=0.0, op0=OP.mult, op1=OP.add)
                        sb = pw1.tile([128, T], f32, tag="sb")
                        nc.vector.tensor_tensor_scan(out=sb, data0=edb, data1=kk,
                                                     initial=0.0, op0=OP.mult, op1=OP.add)
                        # num = eu*kv + shift(S_a); den = eu*k + shift(S_b)
                        nc.vector.tensor_scalar_mul(out=vv, in0=vv, scalar1=pvs(co, V_EU))
                        nc.vector.tensor_add(out=vv[:, 1:T], in0=vv[:, 1:T],
                                             in1=sa[:, 0:T - 1])
                        nc.vector.tensor_scalar_mul(out=kk, in0=kk, scalar1=pvs(co, V_EU))
                        nc.vector.tensor_add(out=kk[:, 1:T], in0=kk[:, 1:T],
                                             in1=sb[:, 0:T - 1])
                        nc.vector.reciprocal(out=kk, in_=kk)
                        nc.vector.tensor_mul(out=vv, in0=vv, in1=kk)   # wkv
                        ab = attbf_p.tile([128, T], bf16, tag="attbf")
                        nc.vector.tensor_mul(out=ab, in0=vv, in1=sr)   # sig(r)*wkv
                        attbf.append(ab)

            # ------------ Phase C: att@Wo^T; x2 = x1+att_out; LN2; mix2 ------
            xm2bf_p = s1.enter_context(tc.tile_pool(name="xm2bf", bufs=NCO))
            xm2bf = []
            with ExitStack() as ph:
                pc1 = ph.enter_context(tc.tile_pool(name="pc1", bufs=1))
                pc2 = ph.enter_context(tc.tile_pool(name="pc2", bufs=2))
                px2 = ph.enter_context(tc.tile_pool(name="px2", bufs=NCO))
                x2t = []
                for co in range(NCO):
                    wow = pc2.tile([128, NCO, 128], bf16, tag="wow")
                    nc.sync.dma_start(out=wow, in_=wo_v[:, :, co * 128:(co + 1) * 128])
                    x1 = pc2.tile([128, T], f32, tag="x1c")
                    nc.sync.dma_start(out=x1, in_=x1t_d[co * 128:(co + 1) * 128, :])
                    x2 = px2.tile([128, T], bf16, tag="x2")
                    for nch in range(NT):
                        ps = pp_mm.tile([128, TCH], f32, tag="mm")
                        for ci in range(NCO):
                            nc.tensor.matmul(
                                ps, wow[:, ci, :],
                                attbf[ci][:, nch * TCH:(nch + 1) * TCH],
                                start=(ci == 0), stop=(ci == NCO - 1))
                        nc.vector.tensor_add(out=x2[:, nch * TCH:(nch + 1) * TCH],
                                             in0=ps, in1=x1[:, nch * TCH:(nch + 1) * TCH])
                    x2t.append(x2)

                # LN2 stats: partition-reduce via ones-matmul (bf16 rhs casts)
                mrow = pc1.tile([1, T], f32, tag="mrow")
                rrow = pc1.tile([1, T], f32, tag="rrow")
                for nch in range(NT):
                    ps = pp_row.tile([1, TCH], f32, tag="rowp")
                    for co in range(NCO):
                        nc.tensor.matmul(ps, ones_col,
                                         x2t[co][:, nch * TCH:(nch + 1) * TCH],
                                         start=(co == 0),
                                         stop=(co == NCO - 1), skip_group_check=True)
                    nc.vector.tensor_scalar_mul(out=mrow[:, nch * TCH:(nch + 1) * TCH],
                                                in0=ps, scalar1=1.0 / C)
                for nch in range(NT):
                    ps = pp_row.tile([1, TCH], f32, tag="rowp")
                    for co in range(NCO):
                        sq = pc2.tile([128, TCH], bf16, tag="sq")
                        nc.scalar.square(out=sq,
                                         in_=x2t[co][:, nch * TCH:(nch + 1) * TCH])
                        nc.tensor.matmul(ps, ones_col, sq, start=(co == 0),
                                         stop=(co == NCO - 1), skip_group_check=True)
                    nc.vector.tensor_scalar_mul(out=rrow[:, nch * TCH:(nch + 1) * TCH],
                                                in0=ps, scalar1=1.0 / C)
                nc.sync.dma_start(out=rows_d[0:1, :], in_=mrow)
                nc.vector.tensor_mul(out=mrow, in0=mrow, in1=mrow)
                nc.vector.tensor_sub(out=rrow, in0=rrow, in1=mrow)
                nc.scalar.activation(out=rrow, in_=rrow, func=AF.Sqrt,
                                     bias=eps_t[0:1, :], scale=1.0)
                nc.vector.reciprocal(out=rrow, in_=rrow)
                nc.sync.dma_start(out=rows_d[1:2, :], in_=rrow)
                mB = pc1.tile([128, T], f32, tag="mB")
                rB = pc1.tile([128, T], f32, tag="rB")
                r0 = rows_d[0:1, :]
                nc.sync.dma_start(out=mB, in_=bass.AP(
                    tensor=r0.tensor, offset=r0.offset, ap=[[0, 128], r0.ap[1]]))
                r1 = rows_d[1:2, :]
                nc.sync.dma_start(out=rB, in_=bass.AP(
                    tensor=r1.tensor, offset=r1.offset, ap=[[0, 128], r1.ap[1]]))

                # x3 = (x2 - m) * rstd * g2 + b2 ; mix2 -> xm2bf ; x3 -> DRAM
                for co in range(NCO):
                    x3 = pc2.tile([128, T], f32, tag="x3")
                    nc.vector.tensor_sub(out=x3, in0=x2t[co], in1=mB)
                    nc.vector.tensor_mul(out=x3, in0=x3, in1=rB)
                    nc.scalar.activation(out=x3, in_=x3, func=AF.Identity,
                                         bias=pvs(co, V_B2), scale=pvs(co, V_G2))
                    nc.sync.dma_start(out=x3t_d[co * 128:(co + 1) * 128, :], in_=x3)
                    xm = pc2.tile([128, T], f32, tag="xm2")
                    nc.vector.tensor_scalar_mul(out=xm, in0=x3, scalar1=pvs(co, V_TMF))
                    nc.vector.scalar_tensor_tensor(
                        out=xm[:, 1:T], in0=x3[:, 0:T - 1], scalar=pvs(co, V_CAF),
                        in1=xm[:, 1:T], op0=OP.mult, op1=OP.add)
                    nc.vector.scalar_tensor_tensor(
                        out=xm[:, 0:T - 1], in0=x3[:, 1:T], scalar=pvs(co, V_CBF),
                        in1=xm[:, 0:T - 1], op0=OP.mult, op1=OP.add)
                    xb = xm2bf_p.tile([128, T], bf16, tag="xm2bf")
                    nc.scalar.copy(out=xb, in_=xm)
                    xm2bf.append(xb)

            # ------------ Phase E: FFN --------------------------------------
            with ExitStack() as ph:
                pe1 = ph.enter_context(tc.tile_pool(name="pe1", bufs=1))
                pe = ph.enter_context(tc.tile_pool(name="pe", bufs=3))
                pk2 = ph.enter_context(tc.tile_pool(name="pk2", bufs=NHO))
                fr_sb = pe1.tile([128, NCO, C], bf16, tag="frw")
                nc.sync.dma_start(out=fr_sb,
                                  in_=fr_d.rearrange("(ci k) m -> k ci m", k=128))
                for tch in range(NT):
                    tsl = slice(tch * TCH, (tch + 1) * TCH)
                    k2 = []
                    for ho in range(NHO):
                        fk_sb = pe.tile([128, NCO, 128], bf16, tag="fkw")
                        nc.sync.dma_start(out=fk_sb,
                                          in_=fk_v[:, :, ho * 128:(ho + 1) * 128])
                        ps = pp_mm.tile([128, TCH], f32, tag="mm")
                        for ci in range(NCO):
                            nc.tensor.matmul(ps, fk_sb[:, ci, :], xm2bf[ci][:, tsl],
                                             start=(ci == 0), stop=(ci == NCO - 1))
                        rl = pe.tile([128, TCH], f32, tag="rl")
                        nc.vector.tensor_scalar_max(out=rl, in0=ps, scalar1=0.0)
                        kb = pk2.tile([128, TCH], bf16, tag="k2")
                        nc.vector.tensor_mul(out=kb, in0=rl, in1=rl)
                        k2.append(kb)
                    for co in range(NCO):
                        psr = pp_mm.tile([128, TCH], f32, tag="mm")
                        for ci in range(NCO):
                            nc.tensor.matmul(psr,
                                             fr_sb[:, ci, co * 128:(co + 1) * 128],
                                             xm2bf[ci][:, tsl],
                                             start=(ci == 0), stop=(ci == NCO - 1))
                        srf = pe.tile([128, TCH], f32, tag="srf")
                        nc.scalar.activation(out=srf, in_=psr, func=AF.Sigmoid)
                        pkv = pp_mm.tile([128, TCH], f32, tag="mm")
                        for ho in range(NHO):
                            fvt = pe.tile([128, 128], bf16, tag="fvt")
                            nc.sync.dma_start(
                                out=fvt,
                                in_=fv_d[ho * 128:(ho + 1) * 128,
                                         co * 128:(co + 1) * 128])
                            nc.tensor.matmul(pkv, fvt, k2[ho], start=(ho == 0),
                                             stop=(ho == NHO - 1))
                        x3c = pe.tile([128, TCH], f32, tag="x3c")
                        nc.sync.dma_start(out=x3c,
                                          in_=x3t_d[co * 128:(co + 1) * 128, tsl])
                        of = pe.tile([128, TCH], f32, tag="of")
                        nc.vector.tensor_mul(out=of, in0=pkv, in1=srf)
                        nc.vector.tensor_add(out=of, in0=of, in1=x3c)
                        # transpose back to token-major and store
                        for bt in range(TCH // 128):
                            pst = pp_tr.tile([128, 128], f32, tag="tra")
                            nc.tensor.transpose(pst, of[:, bt * 128:(bt + 1) * 128],
                                                ident)
                            ot = pe.tile([128, 128], f32, tag="ot")
                            nc.vector.tensor_copy(out=ot, in_=pst)
                            t0 = tch * TCH + bt * 128
                            nc.sync.dma_start(
                                out=y_d[t0:t0 + 128, co * 128:(co + 1) * 128], in_=ot)

    nc.compile()
    return nc


def _prep_inputs(inputs):
    from concourse import mybir
    bf = mybir.dt.np(mybir.dt.bfloat16)
    f = np.float32
    tm = np.asarray(inputs["att_time_mix"], f).reshape(C)
    cm = np.asarray(inputs["att_combined_mix"], f).reshape(C)
    tmf = np.asarray(inputs["ffn_time_mix"], f).reshape(C)
    cmf = np.asarray(inputs["ffn_combined_mix"], f).reshape(C)
    lo = (np.arange(C) < C // 2).astype(f)
    hi = 1.0 - lo
    td = np.asarray(inputs["time_decay"], f)
    tf = np.asarray(inputs["time_first"], f)
    pv = np.stack([
        tm, (1.0 - tm) + cm * lo, cm * hi,
        np.exp(-np.exp(td.astype(np.float64))).astype(f), np.exp(tf),
        np.asarray(inputs["ln1_g"], f), np.asarray(inputs["ln1_b"], f),
        np.asarray(inputs["ln2_g"], f), np.asarray(inputs["ln2_b"], f),
        tmf, (1.0 - tmf) + cmf * lo, cmf * hi,
    ], axis=1).astype(f)                      # [C, 12]
    base = {
        "wk": np.ascontiguousarray(np.asarray(inputs["Wk"], f).T).astype(bf),
        "wv": np.ascontiguousarray(np.asarray(inputs["Wv"], f).T).astype(bf),
        "wr": np.ascontiguousarray(np.asarray(inputs["Wr"], f).T).astype(bf),
        "wo": np.ascontiguousarray(np.asarray(inputs["Wo"], f).T).astype(bf),
        "fk": np.ascontiguousarray(np.asarray(inputs["Fk"], f).T).astype(bf),
        "fv": np.ascontiguousarray(np.asarray(inputs["Fv"], f).T).astype(bf),
        "fr": np.ascontiguousarray(np.asarray(inputs["Fr"], f).T).astype(bf),
        "pv": pv,
    }
    x = np.asarray(inputs["x"], f)
    in_maps = [dict(base, x=np.ascontiguousarray(x[b])) for b in range(B)]
    return in_maps


def kernel(**inputs):
    from concourse.bass_utils import run_bass_kernel_spmd
    if "nc" not in _CACHE:
        _CACHE["nc"] = _build()
    nc = _CACHE["nc"]
    in_maps = _prep_inputs(inputs)
    import tempfile
    kw = {}
    if os.environ.get("BASS_TRACE"):
        kw = dict(trace=True, tmpdir=tempfile.mkdtemp(prefix="rwkv_trace_"))
    res = run_bass_kernel_spmd(nc, in_maps, core_ids=list(range(B)), **kw)
    _CACHE["last_res"] = res
    out = np.stack([res.results[b]["y"] for b in range(B)], axis=0)
    return out.astype(np.float32)



# revision 2
# speedup vs baseline: 1.0244x; 1.0244x over previous
"""RWKV block (LN1 -> time-mix attention w/ WKV scan -> LN2 -> channel-mix FFN)
as a Bass/Tile kernel for 8 Trainium2 NeuronCores — v4.

Sharding: data-parallel over batch B=8 (one batch element per core).
Channel-major [C-partitions, T-free] throughout; host pre-transposes x to
[C,T] and post-transposes y, so the kernel does zero on-chip transposes.
LayerNorm stats via ones-matmul partition reduction + K=1 broadcast matmuls.
Residual adds ride the GEMM accumulation (identity stationary).

v4 = v2 phase structure (full-T per-co WKV, few big DVE ops) + op-level wins:
stride-0 broadcast of the decay column as scan data0 (no edb tiles), fast
approximate reciprocals, scan outputs in [128,T+1] tiles whose leading carry
column keeps the shifted adds aligned, numerator work on GpSimd, and the
token-mix emitted per chunk so it overlaps phase A.
"""
import sys
if '/opt/trn_rl_repo' not in sys.path:
    sys.path.insert(0, '/opt/trn_rl_repo')

import os
import numpy as np

B, T, C = 8, 2048, 1024
H = 4 * C
NCO = C // 128          # 8 channel tiles
NHO = H // 128          # 32 hidden tiles
TCH = 512               # matmul free-dim chunk (one PSUM bank)
NT = T // TCH           # 4 chunks
TH = T // 2             # phase-E half
LN_EPS = 1e-5

# per-channel vector slots in the packed [C, 12] table
(V_A1, V_B1, V_C1, V_ED, V_TF, V_G1, V_BB1, V_G2, V_BB2,
 V_A2, V_B2, V_C2) = range(12)

_CACHE = {}


def _build():
    import concourse.bacc as bacc
    import concourse.tile as tile
    import concourse.bass as bass
    from concourse import mybir
    from concourse.masks import make_identity
    from contextlib import ExitStack

    f32 = mybir.dt.float32
    bf16 = mybir.dt.bfloat16
    AF = mybir.ActivationFunctionType
    OP = mybir.AluOpType

    nc = bacc.Bacc("TRN2", num_devices=B)

    x_d = nc.dram_tensor("x", [C, T], f32, kind="ExternalInput").ap()
    wk_d = nc.dram_tensor("wk", [NCO, 128, C], bf16, kind="ExternalInput").ap()
    wv_d = nc.dram_tensor("wv", [NCO, 128, C], bf16, kind="ExternalInput").ap()
    wr_d = nc.dram_tensor("wr", [NCO, 128, C], bf16, kind="ExternalInput").ap()
    wo_d = nc.dram_tensor("wo", [NCO, 128, C], bf16, kind="ExternalInput").ap()
    fk_d = nc.dram_tensor("fk", [NHO, 128, C], bf16, kind="ExternalInput").ap()
    fv_d = nc.dram_tensor("fv", [NCO, 128, H], bf16, kind="ExternalInput").ap()
    fr_d = nc.dram_tensor("fr", [NCO, 128, C], bf16, kind="ExternalInput").ap()
    pv_d = nc.dram_tensor("pv", [C, 12], f32, kind="ExternalInput").ap()
    y_d = nc.dram_tensor("y", [C, T], f32, kind="ExternalOutput").ap()

    with tile.TileContext(nc) as tc, ExitStack() as top:
        singles = top.enter_context(tc.tile_pool(name="singles", bufs=1))
        ident = singles.tile([128, 128], bf16)
        make_identity(nc, ident)
        ones_col = singles.tile([128, 1], bf16)
        nc.vector.memset(ones_col, 1.0)
        ones_row = singles.tile([1, 128], f32)
        nc.vector.memset(ones_row, 1.0)
        eps_t = singles.tile([1, 1], f32)
        nc.vector.memset(eps_t, LN_EPS)
        pv_sb = []
        for co in range(NCO):
            pvt = singles.tile([128, 12], f32, tag=f"pv{co}")
            nc.sync.dma_start(out=pvt, in_=pv_d[co * 128:(co + 1) * 128, :])
            pv_sb.append(pvt)

        def pvs(co, idx):
            return pv_sb[co][:, idx:idx + 1]

        pp_mm = top.enter_context(tc.tile_pool(name="pp_mm", bufs=3, space="PSUM"))
        pp_row = top.enter_context(tc.tile_pool(name="pp_row", bufs=1, space="PSUM"))
        pp_bc = top.enter_context(tc.tile_pool(name="pp_bc", bufs=1, space="PSUM"))

        def ed_bc(co, n):
            # [128, n] stride-0 broadcast of the per-channel decay column
            base = pvs(co, V_ED)
            return bass.AP(tensor=base.tensor, offset=base.offset,
                           ap=[[base.ap[0][0], 128], [0, n]])

        x3_p = top.enter_context(tc.tile_pool(name="x3_p", bufs=1))
        x3 = []
        for co in range(NCO):
            x3t = x3_p.tile([128, T], bf16, tag=f"x3_{co}")
            x3.append(x3t)
        x1_cm = tc.tile_pool(name="x1_p", bufs=1)
        x1_p = x1_cm.__enter__()
        x1 = []
        for co in range(NCO):
            x1t = x1_p.tile([128, T], bf16, tag=f"x1_{co}")
            x1.append(x1t)

        def ln_rows(x_group, sq_tiles, misc, pp_row, pp_bc):
            """LN stats over partitions for one T-chunk -> (rB, mB) bf16
            SBUF broadcast tiles [128,TCH] of rstd and m*rstd."""
            mps = pp_row.tile([1, TCH], f32, tag="mps")
            for co in range(NCO):
                nc.tensor.matmul(mps, ones_col, x_group[:, co, :],
                                 start=(co == 0), stop=(co == NCO - 1),
                                 skip_group_check=True)
            vps = pp_row.tile([1, TCH], f32, tag="vps")
            for co in range(NCO):
                nc.tensor.matmul(vps, ones_col, sq_tiles[co], start=(co == 0),
                                 stop=(co == NCO - 1), skip_group_check=True)
            m = misc.tile([1, TCH], f32, tag="m")
            nc.vector.tensor_scalar_mul(out=m, in0=mps, scalar1=1.0 / C)
            msq = misc.tile([1, TCH], f32, tag="msq")
            nc.vector.tensor_scalar_mul(out=msq, in0=vps, scalar1=1.0 / C)
            var = misc.tile([1, TCH], f32, tag="var")
            nc.vector.scalar_tensor_tensor(out=var, in0=m, scalar=-1.0,
                                           in1=m, op0=OP.mult, op1=OP.mult)
            nc.vector.tensor_add(out=var, in0=var, in1=msq)
            std = misc.tile([1, TCH], f32, tag="std")
            nc.scalar.activation(out=std, in_=var, func=AF.Sqrt,
                                 bias=eps_t, scale=1.0)
            rstd = misc.tile([1, TCH], f32, tag="rstd")
            nc.vector.reciprocal_approx_fast(out=rstd, in_=std)
            mr = misc.tile([1, TCH], f32, tag="mr")
            nc.vector.tensor_mul(out=mr, in0=m, in1=rstd)
            rstdB = pp_bc.tile([128, TCH], f32, tag="rstdB")
            nc.tensor.matmul(rstdB, ones_row, rstd, start=True, stop=True,
                             skip_group_check=True)
            mrB = pp_bc.tile([128, TCH], f32, tag="mrB")
            nc.tensor.matmul(mrB, ones_row, mr, start=True, stop=True,
                             skip_group_check=True)
            rB = misc.tile([128, TCH], bf16, tag="rB")
            nc.vector.tensor_copy(out=rB, in_=rstdB)
            mB = misc.tile([128, TCH], bf16, tag="mB")
            nc.vector.tensor_copy(out=mB, in_=mrB)
            return rB, mB

        def assemble(dst, src, rB, mB, g_ap, b_ap, tmp_pool, on_gp):
            """dst = (src*rB - mB)*g + b, all bf16 [128,TCH]."""
            eng = nc.gpsimd if on_gp else nc.vector
            t1 = tmp_pool.tile([128, TCH], bf16, tag="t1g" if on_gp else "t1v")
            eng.tensor_mul(out=t1, in0=src, in1=rB)
            eng.tensor_sub(out=t1, in0=t1, in1=mB)
            nc.vector.tensor_scalar(out=dst, in0=t1, scalar1=g_ap,
                                    scalar2=b_ap, op0=OP.mult, op1=OP.add)

        def mix_chunk(xg, co, src, tch, a_i, b_i, c_i, d0=0):
            """Per-chunk token mix: xm = A*x + B*shift(x) + C*antishift(x).
            Writes xg[:, co, d0:d0+TCH] from src[:, t0:t0+TCH] (+/-1 taps)."""
            t0 = tch * TCH
            nc.vector.tensor_scalar_mul(out=xg[:, co, d0:d0 + TCH],
                                        in0=src[:, t0:t0 + TCH],
                                        scalar1=pvs(co, a_i))
            lo = 1 if tch == 0 else 0
            nc.vector.scalar_tensor_tensor(
                out=xg[:, co, d0 + lo:d0 + TCH],
                in0=src[:, t0 + lo - 1:t0 + TCH - 1],
                scalar=pvs(co, b_i), in1=xg[:, co, d0 + lo:d0 + TCH],
                op0=OP.mult, op1=OP.add)
            hi = TCH - 1 if tch == NT - 1 else TCH
            nc.vector.scalar_tensor_tensor(
                out=xg[:, co, d0:d0 + hi], in0=src[:, t0 + 1:t0 + hi + 1],
                scalar=pvs(co, c_i), in1=xg[:, co, d0:d0 + hi],
                op0=OP.mult, op1=OP.add)

        # ---------------- Phase A: LN1 (channel-major) -> x1 bf16 ----------
        with ExitStack() as ph:
            pa_grp = ph.enter_context(tc.tile_pool(name="pa_grp", bufs=2))
            pa_x = ph.enter_context(tc.tile_pool(name="pa_x", bufs=3))
            pa_sq = ph.enter_context(tc.tile_pool(name="pa_sq", bufs=NCO + 2))
            pa_ms = ph.enter_context(tc.tile_pool(name="pa_ms", bufs=2))
            pa_t = ph.enter_context(tc.tile_pool(name="pa_t", bufs=3))
            for tch in range(NT):
                tsl = slice(tch * TCH, (tch + 1) * TCH)
                xg = pa_grp.tile([128, NCO, TCH], bf16, tag="xg")
                sqs = []
                for co in range(NCO):
                    xf = pa_x.tile([128, TCH], f32, tag="xf")
                    nc.sync.dma_start(out=xf, in_=x_d[co * 128:(co + 1) * 128, tsl])
                    nc.scalar.copy(out=xg[:, co, :], in_=xf)
                    sq = pa_sq.tile([128, TCH], bf16, tag="sq")
                    nc.scalar.square(out=sq, in_=xf)
                    sqs.append(sq)
                rB, mB = ln_rows(xg, sqs, pa_ms, pp_row, pp_bc)
                for co in range(NCO):
                    assemble(x1[co][:, tsl], xg[:, co, :], rB, mB,
                             pvs(co, V_G1), pvs(co, V_BB1), pa_t, on_gp=(co >= 6))

        with ExitStack() as sBC:
            att_cm = tc.tile_pool(name="att_p", bufs=1)
            att_p = att_cm.__enter__()
            ATT = att_p.tile([128, NCO, T], bf16)

            # ---------- Phase B: mix -> k/v/r GEMMs -> WKV scan -> ATT -----
            with ExitStack() as ph:
                xm_p = ph.enter_context(tc.tile_pool(name="xm_p", bufs=1))
                XMc = []
                for tch in range(NT):
                    xmt = xm_p.tile([128, NCO, TCH], bf16, tag=f"xm{tch}")
                    XMc.append(xmt)
                # chunked so mix overlaps the tail of phase A and B can
                # start before all chunks are mixed (tile-granular deps)
                for tch in range(NT):
                    for co in range(NCO):
                        mix_chunk(XMc[tch], co, x1[co], tch, V_A1, V_B1, V_C1)

                pw = ph.enter_context(tc.tile_pool(name="pw", bufs=2))
                pb2a = ph.enter_context(tc.tile_pool(name="pb2a", bufs=2))
                pbn = ph.enter_context(tc.tile_pool(name="pbn", bufs=2))
                pbz = ph.enter_context(tc.tile_pool(name="pbz", bufs=1))
                pb1 = ph.enter_context(tc.tile_pool(name="pb1", bufs=1))
                for co in range(NCO):
                    wkw = pw.tile([128, C], bf16, tag="wkw")
                    nc.sync.dma_start(out=wkw, in_=wk_d[co])
                    wvw = pw.tile([128, C], bf16, tag="wvw")
                    nc.sync.dma_start(out=wvw, in_=wv_d[co])
                    wrw = pw.tile([128, C], bf16, tag="wrw")
                    nc.sync.dma_start(out=wrw, in_=wr_d[co])

                    kk = pb2a.tile([128, T], bf16, tag="kk")
                    keu = pb2a.tile([128, T], bf16, tag="keu")
                    vv = pb2a.tile([128, T], bf16, tag="vv")
                    tr = pb2a.tile([128, T], bf16, tag="tr")
                    for nch in range(NT):
                        nsl = slice(nch * TCH, (nch + 1) * TCH)
                        ps = pp_mm.tile([128, TCH], f32, tag="mm")
                        for ci in range(NCO):
                            nc.tensor.matmul(
                                ps, wkw[:, ci * 128:(ci + 1) * 128],
                                XMc[nch][:, ci, :],
                                start=(ci == 0), stop=(ci == NCO - 1))
                        nc.scalar.activation(out=kk[:, nsl], in_=ps, func=AF.Exp)
                        nc.scalar.activation(out=keu[:, nsl], in_=ps, func=AF.Exp,
                                             bias=pvs(co, V_TF), scale=1.0)
                    for nch in range(NT):
                        nsl = slice(nch * TCH, (nch + 1) * TCH)
                        ps = pp_mm.tile([128, TCH], f32, tag="mm")
                        for ci in range(NCO):
                            nc.tensor.matmul(
                                ps, wvw[:, ci * 128:(ci + 1) * 128],
                                XMc[nch][:, ci, :],
                                start=(ci == 0), stop=(ci == NCO - 1))
                        nc.scalar.copy(out=vv[:, nsl], in_=ps)
                    for nch in range(NT):
                        nsl = slice(nch * TCH, (nch + 1) * TCH)
                        ps = pp_mm.tile([128, TCH], f32, tag="mm")
                        for ci in range(NCO):
                            nc.tensor.matmul(
                                ps, wrw[:, ci * 128:(ci + 1) * 128],
                                XMc[nch][:, ci, :],
                                start=(ci == 0), stop=(ci == NCO - 1))
                        nc.scalar.activation(out=tr[:, nsl], in_=ps,
                                             func=AF.Tanh, scale=0.5)

                    # WKV: y = (eu*k*v + sa_{t-1}) / (eu*k + sb_{t-1})
                    ncur = pbn.tile([128, T], bf16, tag="ncur")
                    nc.vector.tensor_mul(out=ncur, in0=keu, in1=vv)
                    nc.vector.tensor_mul(out=vv, in0=kk, in1=vv)  # vv := k*v
                    saz = pbz.tile([128, T + 1], bf16, tag="saz")
                    nc.vector.memset(saz[:, 0:1], 0.0)
                    nc.vector.tensor_tensor_scan(
                        out=saz[:, 1:T + 1], data0=ed_bc(co, T), data1=vv,
                        initial=0.0, op0=OP.mult, op1=OP.add)
                    sbz = pbz.tile([128, T + 1], bf16, tag="sbz")
                    nc.vector.memset(sbz[:, 0:1], 0.0)
                    nc.vector.tensor_tensor_scan(
                        out=sbz[:, 1:T + 1], data0=ed_bc(co, T), data1=kk,
                        initial=0.0, op0=OP.mult, op1=OP.add)
                    nc.vector.tensor_add(out=ncur, in0=ncur, in1=saz[:, 0:T])
                    den = pb1.tile([128, T], f32, tag="den")
                    nc.vector.tensor_add(out=den, in0=keu, in1=sbz[:, 0:T])
                    nc.vector.reciprocal_approx_fast(out=den, in_=den)
                    nc.vector.tensor_mul(out=ncur, in0=ncur, in1=den)
                    nc.vector.scalar_tensor_tensor(
                        out=ATT[:, co, :], in0=tr, scalar=1.0, in1=ncur,
                        op0=OP.add, op1=OP.mult)

            # ---------- Phase C: Wo GEMM + residual; LN2 -> x3 -------------
            with ExitStack() as ph:
                pc_grp = ph.enter_context(tc.tile_pool(name="pc_grp", bufs=1))
                X2 = pc_grp.tile([128, NCO, T], bf16)
                pw = ph.enter_context(tc.tile_pool(name="pcw", bufs=1))
                pc_sq = ph.enter_context(tc.tile_pool(name="pc_sq", bufs=NCO + 1))
                pc_ms = ph.enter_context(tc.tile_pool(name="pc_ms", bufs=1))
                pc_t = ph.enter_context(tc.tile_pool(name="pc_t", bufs=2))
                wow_all = []
                for co in range(NCO):
                    wow = pw.tile([128, C], bf16, tag=f"wow{co}")
                    nc.sync.dma_start(out=wow, in_=wo_d[co])
                    wow_all.append(wow)
                for tch in range(NT):
                    tsl = slice(tch * TCH, (tch + 1) * TCH)
                    sqs = []
                    for co in range(NCO):
                        ps = pp_mm.tile([128, TCH], f32, tag="mm")
                        for ci in range(NCO):
                            nc.tensor.matmul(
                                ps, wow_all[co][:, ci * 128:(ci + 1) * 128],
                                ATT[:, ci, tsl],
                                start=(ci == 0), stop=False)
                        nc.tensor.matmul(ps, ident, x1[co][:, tsl],
                                         start=False, stop=True)
                        nc.scalar.copy(out=X2[:, co, tsl], in_=ps)
                        sq = pc_sq.tile([128, TCH], bf16, tag="sqc")
                        nc.scalar.square(out=sq, in_=ps)
                        sqs.append(sq)
                    rB, mB = ln_rows(X2[:, :, tsl], sqs, pc_ms, pp_row, pp_bc)
                    for co in range(NCO):
                        assemble(x3[co][:, tsl], X2[:, co, tsl], rB, mB,
                                 pvs(co, V_G2), pvs(co, V_BB2), pc_t,
                                 on_gp=(co >= 6))

            att_cm.__exit__(None, None, None)
            x1_cm.__exit__(None, None, None)
            xm2_p = sBC.enter_context(tc.tile_pool(name="xm2_p", bufs=1))
            XM2c = []
            for tch in range(NT):
                xmt = xm2_p.tile([128, NCO, TCH], bf16, tag=f"xm2_{tch}")
                XM2c.append(xmt)
            for tch in range(NT):
                for co in range(NCO):
                    mix_chunk(XM2c[tch], co, x3[co], tch, V_A2, V_B2, V_C2)

            # ---------- Phase E: FFN ---------------------------------------
            with ExitStack() as ph:
                pk2 = ph.enter_context(tc.tile_pool(name="pk2", bufs=1))
                k2h = pk2.tile([128, NHO, TH], bf16)
                pfk = ph.enter_context(tc.tile_pool(name="pfk", bufs=3))
                pfv = ph.enter_context(tc.tile_pool(name="pfv", bufs=2))
                pfr = ph.enter_context(tc.tile_pool(name="pfr", bufs=2))
                pe_t = ph.enter_context(tc.tile_pool(name="pe_t", bufs=3))
                pe_y = ph.enter_context(tc.tile_pool(name="pe_y", bufs=3))
                NH2 = TH // TCH  # chunks per half
                for half in range(2):
                    for ho in range(NHO):
                        fkw = pfk.tile([128, C], bf16, tag="fkw")
                        nc.sync.dma_start(out=fkw, in_=fk_d[ho])
                        for nch in range(NH2):
                            t0 = half * TH + nch * TCH
                            nsl = slice(t0, t0 + TCH)
                            ps = pp_mm.tile([128, TCH], f32, tag="mm")
                            for ci in range(NCO):
                                nc.tensor.matmul(
                                    ps, fkw[:, ci * 128:(ci + 1) * 128],
                                    XM2c[t0 // TCH][:, ci, :],
                                    start=(ci == 0), stop=(ci == NCO - 1))
                            rl = pe_t.tile([128, TCH], bf16, tag="rl")
                            nc.scalar.activation(out=rl, in_=ps, func=AF.Relu)
                            nc.gpsimd.tensor_mul(
                                out=k2h[:, ho, nch * TCH:(nch + 1) * TCH],
                                in0=rl, in1=rl)
                    for co in range(NCO):
                        fvw = pfv.tile([128, H], bf16, tag="fvw")
                        nc.sync.dma_start(out=fvw, in_=fv_d[co])
                        frw = pfr.tile([128, C], bf16, tag="frw")
                        nc.sync.dma_start(out=frw, in_=fr_d[co])
                        for nch in range(NH2):
                            t0 = half * TH + nch * TCH
                            nsl = slice(t0, t0 + TCH)
                            ksl = slice(nch * TCH, (nch + 1) * TCH)
                            psr = pp_mm.tile([128, TCH], f32, tag="mm")
                            for ci in range(NCO):
                                nc.tensor.matmul(
                                    psr, frw[:, ci * 128:(ci + 1) * 128],
                                    XM2c[t0 // TCH][:, ci, :],
                                    start=(ci == 0), stop=(ci == NCO - 1))
                            trf = pe_t.tile([128, TCH], bf16, tag="trf")
                            nc.scalar.activation(out=trf, in_=psr,
                                                 func=AF.Tanh, scale=0.5)
                            pkv = pp_mm.tile([128, TCH], f32, tag="mm")
                            for ho in range(NHO):
                                nc.tensor.matmul(
                                    pkv, fvw[:, ho * 128:(ho + 1) * 128],
                                    k2h[:, ho, ksl],
                                    start=(ho == 0), stop=(ho == NHO - 1))
                            kvs = pe_t.tile([128, TCH], bf16, tag="kvs")
                            nc.scalar.copy(out=kvs, in_=pkv)
                            y1 = pe_t.tile([128, TCH], bf16, tag="ey1")
                            nc.vector.scalar_tensor_tensor(
                                out=y1, in0=trf, scalar=1.0, in1=kvs,
                                op0=OP.add, op1=OP.mult)
                            yf = pe_y.tile([128, TCH], f32, tag="yf")
                            nc.vector.tensor_add(out=yf, in0=y1,
                                                 in1=x3[co][:, nsl])
                            nc.sync.dma_start(
                                out=y_d[co * 128:(co + 1) * 128, nsl], in_=yf)

    nc.compile()
    return nc


def _prep_inputs(inputs):
    from concourse import mybir
    bf = mybir.dt.np(mybir.dt.bfloat16)
    f = np.float32

    def watt(w, scale=1.0):
        # W [out, in] -> lhsT layout [co, k, ci*128+m] for out = W @ x
        wT = np.asarray(w, f).T * scale              # [in, out]
        cin, cout = wT.shape
        a = wT.reshape(cin // 128, 128, cout // 128, 128)  # [ci, k, co, m]
        a = np.ascontiguousarray(a.transpose(2, 1, 0, 3))  # [co, k, ci, m]
        return a.reshape(cout // 128, 128, cin).astype(bf)

    tm = np.asarray(inputs["att_time_mix"], f).reshape(C)
    cm = np.asarray(inputs["att_combined_mix"], f).reshape(C)
    tmf = np.asarray(inputs["ffn_time_mix"], f).reshape(C)
    cmf = np.asarray(inputs["ffn_combined_mix"], f).reshape(C)
    lo = (np.arange(C) < C // 2).astype(f)
    hi = 1.0 - lo
    td = np.asarray(inputs["time_decay"], f)
    tf = np.asarray(inputs["time_first"], f)
    pv = np.stack([
        tm, (1.0 - tm) + cm * lo, cm * hi,
        np.exp(-np.exp(td.astype(np.float64))).astype(f), tf,
        np.asarray(inputs["ln1_g"], f), np.asarray(inputs["ln1_b"], f),
        np.asarray(inputs["ln2_g"], f), np.asarray(inputs["ln2_b"], f),
        tmf, (1.0 - tmf) + cmf * lo, cmf * hi,
    ], axis=1).astype(f)                      # [C, 12]
    base = {
        "wk": watt(inputs["Wk"]),
        "wv": watt(inputs["Wv"]),
        "wr": watt(inputs["Wr"]),
        "wo": watt(inputs["Wo"], 0.5),
        "fk": watt(inputs["Fk"]),
        "fv": watt(inputs["Fv"], 0.5),
        "fr": watt(inputs["Fr"]),
        "pv": pv,
    }
    x = np.asarray(inputs["x"], f)
    in_maps = [dict(base, x=np.ascontiguousarray(x[b].T)) for b in range(B)]
    return in_maps


def kernel(**inputs):
    from concourse.bass_utils import run_bass_kernel_spmd
    if "nc" not in _CACHE:
        _CACHE["nc"] = _build()
    nc = _CACHE["nc"]
    in_maps = _prep_inputs(inputs)
    import tempfile
    kw = {}
    if os.environ.get("BASS_TRACE"):
        kw = dict(trace=True, tmpdir=tempfile.mkdtemp(prefix="rwkv_trace_"))
    res = run_bass_kernel_spmd(nc, in_maps, core_ids=list(range(B)), **kw)
    _CACHE["last_res"] = res
    out = np.stack([np.ascontiguousarray(res.results[b]["y"].T)
                    for b in range(B)], axis=0)
    return out.astype(np.float32)
